# revision 1
# baseline (speedup 1.0000x reference)
"""Trainium2 Bass kernel for CirculatePairConLoss.

Reference math (N=4096, D=64, C=16, T=0.05):
    feats = concat(f1, f2)                  # [2N, D]
    sim   = exp(feats @ feats.T / T)        # [2N, 2N]
    Ng_i  = sum_{j: lab_j != lab_i} sim_ij
    pos_i = exp(<f1_i, f2_i> / T)           (duplicated for both halves)
    term  = -log(pos / (Ng + pos))
    loss  = sum(term / group_size),  group_size_i = 2 * count(label == lab_i)

v3 device strategy (8 cores, SPMD, full I/O):
  Rows sorted by label.  The upper triangle of the [16x16] grid of 512-wide
  blocks (136 blocks) is cut into per-block-row column SEGMENTS of width
  <=3 blocks; segments are dealt so every core runs the identical program:
  7 groups with widths [3,3,3,3,2,2,1] (17 blocks, 68 [128x512] subtiles).

  Per group g (block-row r, cols c0..c0+w), per strip-pair stp in {0,1}:
    * logits for strips 2stp/2stp+1 run as K=64 ROW-TILED matmul pairs
      (tile_position (0,0)/(64,0)) -- 2 strips concurrently on the PE,
    * one exp ACT per strip over the [128, w*512] PSUM chunk, bf16 out,
      with accum_out producing that strip's row-total partial for free,
    * per-class column sums via COL-TILED one-hot matmuls: the w column
      blocks accumulate into one PSUM bank at partition offsets 32q
      (concurrent on the PE; fp32 accumulation over the 4 strips),
    * same-class row partials for near-diagonal blocks via one fused DVE
      scalar_tensor_tensor (is_equal -> mult -> row-sum accum) against
      replicated column labels -- mask tensors never materialize.
  Groups 0,1 hold the (<=2) "row-first" triads of each block-row (their
  q=1,2 blocks are the near-diagonal masked ones), group 4 holds masked
  pairs; unused mask slots get label -2 (matches nothing).

  Outputs per core: per-class col sums (acc), per-strip row totals (tot),
  masked same partials, f1.f2 dots.  Host does an O(2N*C) gather + log.
"""

import numpy as np
import ml_dtypes

import concourse.bass as bass
import concourse.tile as tile
from concourse import bacc, mybir
from concourse.bass_utils import run_bass_kernel_spmd

N = 4096
D = 64
C = 16
TWO_N = 2 * N
TEMP = 0.05
SCALE = 1.0 / TEMP  # 20.0
NCORES = 8

BLK = 512                      # column block width
JSUB = 128                     # row strip height
NBLK = TWO_N // BLK            # 16
W = [3, 3, 3, 3, 2, 2, 1]      # group widths (universal program)
NG = len(W)                    # 7 groups
NBLOCKS = sum(W)               # 17 block-slots
BOFF = np.cumsum([0] + W)      # block-slot offset per group
MASK_GROUPS = {0: 0, 1: 1, 4: 2}   # group -> mask slot
NMS = 3                        # mask slots
I_PER_CORE = TWO_N // NCORES   # 1024 (f1.f2 dots per core)

BF16 = mybir.dt.bfloat16
F32 = mybir.dt.float32

# column-segment widths per block-row size (first segment starts at the
# diagonal and carries the near-diagonal masked blocks)
SIZE_PARTS = {
    16: [3, 3, 3, 3, 3, 1], 15: [3, 3, 3, 3, 2, 1], 14: [3, 3, 3, 2, 2, 1],
    13: [3, 3, 3, 3, 1], 12: [3, 3, 3, 2, 1], 11: [3, 3, 3, 2],
    10: [3, 3, 2, 2], 9: [3, 3, 2, 1], 8: [3, 3, 2], 7: [3, 2, 2],
    6: [3, 2, 1], 5: [3, 2], 4: [2, 2], 3: [3], 2: [2], 1: [1],
}

_CACHE = {}


def _deal_segments():
    """Global segment list -> per-core slot assignment [T,T,T,T,P,P,S]."""
    first_triads, far_triads, pairs_masked, pairs_far, singles = [], [], [], [], []
    for r in range(NBLK):
        size = NBLK - r
        c0 = r
        for i, w in enumerate(SIZE_PARTS[size]):
            seg = (r, c0, w)
            has_mask = any(c0 + q > r and c0 + q <= r + 2 for q in range(w))
            if w == 3:
                (first_triads if i == 0 else far_triads).append(seg)
            elif w == 2:
                (pairs_masked if has_mask else pairs_far).append(seg)
            else:
                singles.append(seg)
            c0 += w
    assert len(first_triads) == 13 and len(far_triads) == 19
    assert len(pairs_masked) == 3 and len(pairs_far) == 13
    assert len(singles) == 8

    cores = []
    ft, fa, pm, pf, sg = (list(first_triads), list(far_triads),
                          list(pairs_masked), list(pairs_far), list(singles))
    for k in range(NCORES):
        slots = []
        # slots 0,1: triads; prefer first-triads (they carry the masks)
        for _ in range(2):
            slots.append(ft.pop(0) if ft else fa.pop(0))
        # slots 2,3: far triads
        slots.append(fa.pop(0))
        slots.append(fa.pop(0))
        # slot 4: masked pair if any left, else far pair
        slots.append(pm.pop(0) if pm else pf.pop(0))
        # slot 5: far pair
        slots.append(pf.pop(0))
        # slot 6: single
        slots.append(sg.pop(0))
        cores.append(slots)
    assert not (ft or fa or pm or pf or sg)
    return cores


# stt windows: triad slots cover sim cols [512:1152] (blocks q1 + first
# 128 of q2); the pair slot covers [0:1024].
MS_OFF = [0, 640, 1280]        # labrep column offset per mask slot
MS_WIN = [640, 640, 1024]
LABREP_W = 2304
# exp chunks computed on DVE via the Schraudolph int16 trick instead of
# ScalarE (load balance): groups 5 (pair) and 6 (single)
# all exp chunks stay on ScalarE: with the 2-deep PSUM logits pool the
# pipeline advances at the per-chunk max over engines, and a DVE
# Schraudolph chunk (~1.7us) costs more than any Scalar ACT chunk
DVE_EXP = set()
# g6's row totals come from a DVE reduce instead of ACT accum_out -- its
# narrow ACTs lose more to the READ_ACCUMULATOR than the DVE pays
DVE_TOT = {(6, st) for st in range(4)}
# smallest group first: its inputs are tiny, so compute starts ~4us
# earlier while the big feature groups stream in behind
GROUP_ORDER = [6, 5, 4, 0, 1, 2, 3]
STEP_ORDER = [(g, sp) for g in GROUP_ORDER for sp in range(2)]
FP8 = mybir.dt.float8e4
FP8_AMP = 16.0   # features pre-scaled into fp8e4m3's normal range
LOG2E = 1.4426950408889634
ACT_SCALE = SCALE / (16.0 * 16.0)   # fp8 features carry a x16 pre-scale
SCH_A16 = 128.0 * LOG2E * ACT_SCALE
SCH_B16 = 128.0 * (127.0 - 0.05744)
I16 = mybir.dt.int16


def _build_v3():
    nc = bacc.Bacc("TRN2", target_bir_lowering=False, debug=False,
                   num_devices=NCORES)

    ftl = nc.declare_dram_parameter("ftl", [128, NG * 2 * 128], FP8, isOutput=False)
    ftr = nc.declare_dram_parameter("ftr", [128, NBLOCKS * BLK], FP8, isOutput=False)
    hx = nc.declare_dram_parameter("hx", [128, NG * 4 * C], BF16, isOutput=False)
    labrep = nc.declare_dram_parameter("labrep", [128, LABREP_W], BF16, isOutput=False)
    labrow = nc.declare_dram_parameter("labrow", [128, NMS * 4], F32, isOutput=False)
    a_i = nc.declare_dram_parameter("a_i", [128, 8 * 64], BF16, isOutput=False)
    b_i = nc.declare_dram_parameter("b_i", [128, 8 * 64], BF16, isOutput=False)

    acc_out = nc.declare_dram_parameter("acc_out", [96, NG * BLK], F32, isOutput=True)
    tot_out = nc.declare_dram_parameter("tot_out", [128, NG * 4], F32, isOutput=True)
    same_out = nc.declare_dram_parameter("same_out", [128, NMS * 4], F32, isOutput=True)
    dots_out = nc.declare_dram_parameter("dots_out", [128, 8], F32, isOutput=True)

    EXP = mybir.ActivationFunctionType.Exp

    with tile.TileContext(nc) as tc:
        with (
            tc.tile_pool(name="consts", bufs=1) as consts,
            tc.tile_pool(name="simpool", bufs=3) as simpool,
            tc.tile_pool(name="small", bufs=2) as small,
            tc.tile_pool(name="plog", bufs=2, space="PSUM") as plog,
            tc.tile_pool(name="pacc", bufs=2, space="PSUM") as pacc,
        ):
            # ---- inputs: per-group tensors DMA'd in GROUP_ORDER,
            # round-robined over the three DMA-capable rings, so the first
            # (smallest) group's deps land ~1.5us after the preamble and
            # compute starts while everything else streams in behind
            ftl_sb = {}
            ftr_sb = {}
            hx_sb = {}
            labrep_sb = consts.tile([128, LABREP_W], BF16)
            labrow_sb = consts.tile([128, NMS * 4], F32)
            a_sb = consts.tile([128, 8 * 64], BF16)
            b_sb = consts.tile([128, 8 * 64], BF16)
            rings = [nc.gpsimd, nc.sync, nc.scalar]
            for i, g in enumerate(GROUP_ORDER):
                tl = consts.tile([128, 2 * 128], FP8, name=f"ftl{g}")
                rings[(3 * i) % 3].dma_start(
                    out=tl, in_=ftl[:, (g * 2) * 128:(g * 2 + 2) * 128])
                tr = consts.tile([128, W[g] * BLK], FP8, name=f"ftr{g}")
                sl = slice(BOFF[g] * BLK, (BOFF[g] + W[g]) * BLK)
                rings[(3 * i + 1) % 3].dma_start(out=tr, in_=ftr[:, sl])
                th = consts.tile([128, 4 * C], BF16, name=f"hx{g}")
                rings[(3 * i + 2) % 3].dma_start(
                    out=th, in_=hx[:, (g * 4) * C:(g * 4 + 4) * C])
                ftl_sb[g], ftr_sb[g], hx_sb[g] = tl, tr, th
                if i == 2:
                    # mask labels: first stt (g0) needs them mid-stream
                    nc.gpsimd.dma_start(out=labrep_sb, in_=labrep[:])
                    nc.sync.dma_start(out=labrow_sb, in_=labrow[:])
            nc.gpsimd.dma_start(out=a_sb, in_=a_i[:])
            nc.sync.dma_start(out=b_sb, in_=b_i[:])
            ones_sb = consts.tile([64, 1], BF16)
            nc.vector.memset(ones_sb, 1.0)

            acc_sb = consts.tile([96, NG * BLK], F32)
            tot_sb = consts.tile([128, NG * 4], F32)
            same_sb = consts.tile([128, NMS * 4], F32)
            dots_sb = consts.tile([128, 8], F32)
            sink = consts.tile([128, 1024], BF16)
            dsink = consts.tile([128, 64], F32)
            warm = consts.tile([64, 1], F32)

            # load the exp table set while input DMAs stream
            nc.scalar.activation(out=warm, in_=ones_sb, func=EXP, scale=0.0)

            # ---- main loop: flat strip-pair steps in STEP_ORDER.  Each
            # step's two logits matmul sets run row-tiled (K=64 halves of
            # the PE at (0,0)/(64,0), concurrent).  H-matmuls are delayed
            # one half-step so the tensor queue never head-blocks on the
            # current exp (the next logits go first).
            acc_ps = {}
            pending = []
            for sidx, (g, sp) in enumerate(STEP_ORDER):
                w = W[g]
                if sp == 0:
                    acc_ps[g] = pacc.tile([128, BLK], F32, tag="acc",
                                          name=f"acc{g}")
                fsl = sp * 128
                lg = [plog.tile([128, w * BLK], F32, tag="lg",
                                name=f"lg{h}") for h in range(2)]
                for q in range(w):
                    qsl = slice(q * BLK, (q + 1) * BLK)
                    nc.tensor.matmul(
                        lg[0][:, qsl], ftl_sb[g][0:64, fsl:fsl + 128],
                        ftr_sb[g][0:64, qsl],
                        start=True, stop=True, tile_position=(0, 0))
                    nc.tensor.matmul(
                        lg[1][:, qsl], ftl_sb[g][64:128, fsl:fsl + 128],
                        ftr_sb[g][64:128, qsl],
                        start=True, stop=True, tile_position=(64, 0))
                for pend in pending:
                    pend()
                pending = []
                for half in range(2):
                    st = 2 * sp + half
                    ti = g * 4 + st
                    if (g, st) in DVE_EXP:
                        s16 = simpool.tile([128, w * BLK], I16, tag="sim")
                        nc.vector.tensor_scalar(
                            out=s16, in0=lg[half], scalar1=SCH_A16,
                            scalar2=SCH_B16, op0=mybir.AluOpType.mult,
                            op1=mybir.AluOpType.add)
                        sim = s16.bitcast(BF16)
                        nc.vector.reduce_sum(
                            out=tot_sb[:, ti:ti + 1], in_=sim,
                            axis=mybir.AxisListType.X)
                    elif (g, st) in DVE_TOT:
                        sim = simpool.tile([128, w * BLK], BF16, tag="sim")
                        nc.scalar.activation(
                            out=sim, in_=lg[half], func=EXP,
                            scale=ACT_SCALE)
                        nc.vector.reduce_sum(
                            out=tot_sb[:, ti:ti + 1], in_=sim,
                            axis=mybir.AxisListType.X)
                    else:
                        sim = simpool.tile([128, w * BLK], BF16, tag="sim")
                        nc.scalar.activation(
                            out=sim, in_=lg[half], func=EXP,
                            scale=ACT_SCALE,
                            accum_out=tot_sb[:, ti:ti + 1])

                    def mk_pending(g=g, st=st, ti=ti, w=w, sim=sim):
                        def emit():
                            for q in range(w):
                                nc.tensor.matmul(
                                    acc_ps[g][32 * q:32 * q + 16, :],
                                    hx_sb[g][:, st * C:(st + 1) * C],
                                    sim[:, q * BLK:(q + 1) * BLK],
                                    start=(st == 0), stop=(st == 3),
                                    tile_position=(0, 32 * q))
                            if g in MASK_GROUPS:
                                ms = MASK_GROUPS[g]
                                lo = BLK if w == 3 else 0
                                si = ms * 4 + st
                                nc.vector.scalar_tensor_tensor(
                                    out=sink[:, 0:MS_WIN[ms]],
                                    in0=labrep_sb[:, MS_OFF[ms]:MS_OFF[ms] + MS_WIN[ms]],
                                    scalar=labrow_sb[:, si:si + 1],
                                    in1=sim[:, lo:lo + MS_WIN[ms]],
                                    op0=mybir.AluOpType.is_equal,
                                    op1=mybir.AluOpType.mult,
                                    accum_out=same_sb[:, si:si + 1])
                            if st == 3:
                                nc.vector.tensor_copy(
                                    acc_sb[:, g * BLK:(g + 1) * BLK],
                                    acc_ps[g][0:96, :])
                                nc.gpsimd.dma_start(
                                    out=acc_out[:, g * BLK:(g + 1) * BLK],
                                    in_=acc_sb[:, g * BLK:(g + 1) * BLK])
                        return emit
                    pending.append(mk_pending())
                # f1.f2 dots: one fused multiply-reduce per 128 rows,
                # sprinkled into the step stream (DVE, no PSUM needed)
                if 6 <= sidx < 10:
                    for t in (2 * (sidx - 6), 2 * (sidx - 6) + 1):
                        nc.vector.scalar_tensor_tensor(
                            out=dsink,
                            in0=a_sb[:, t * 64:(t + 1) * 64],
                            scalar=1.0,
                            in1=b_sb[:, t * 64:(t + 1) * 64],
                            op0=mybir.AluOpType.mult,
                            op1=mybir.AluOpType.mult,
                            accum_out=dots_sb[:, t:t + 1])
            for pend in pending:
                pend()

            nc.sync.dma_start(out=tot_out[:], in_=tot_sb)
            nc.sync.dma_start(out=same_out[:], in_=same_sb)
            nc.sync.dma_start(out=dots_out[:], in_=dots_sb)

    nc.compile()
    return nc


def _kernel_v3(f1, f2, label):
    if "nc3" not in _CACHE:
        _CACHE["nc3"] = _build_v3()
    nc = _CACHE["nc3"]

    feats = np.concatenate([f1, f2], axis=0)
    lab2 = np.concatenate([label, label], axis=0)
    perm = np.argsort(lab2, kind="stable")
    labs = lab2[perm]
    fsT = np.ascontiguousarray(feats[perm].T)          # [D, 2N] f32 sorted
    fsT_f8 = (fsT * FP8_AMP).astype(ml_dtypes.float8_e4m3)

    # each class, starting in block r, must end within the 640-wide stt
    # window of block r's strips: max extent r*512 + 1152
    for c in range(C):
        idx = np.where(labs == c)[0]
        if idx.size and idx[-1] >= (idx[0] // BLK) * BLK + 1152:
            raise _FallbackToV1()

    eye = np.eye(C, dtype=np.float32)
    hot = eye[labs].astype(ml_dtypes.bfloat16)         # [2N, C]
    labs_f = labs.astype(np.float32)
    labs_bf = labs_f.astype(ml_dtypes.bfloat16)
    f1_bf = f1.astype(ml_dtypes.bfloat16)
    f2_bf = f2.astype(ml_dtypes.bfloat16)

    cores = _deal_segments()
    in_maps = []
    for k in range(NCORES):
        segs = cores[k]
        ftl = np.zeros((128, NG * 2 * 128), dtype=ml_dtypes.float8_e4m3)
        ftrp = np.zeros((128, NBLOCKS * BLK), dtype=ml_dtypes.float8_e4m3)
        hx = np.zeros((128, NG * 4 * C), dtype=ml_dtypes.bfloat16)
        labrep = np.full((128, LABREP_W), -2.0, dtype=ml_dtypes.bfloat16)
        labrow = np.zeros((128, NMS * 4), dtype=np.float32)
        for g, (r, c0, w) in enumerate(segs):
            for st in range(4):
                x0 = r * BLK + st * 128
                half = slice(0, 64) if st % 2 == 0 else slice(64, 128)
                sp = st // 2
                ftl[half, (g * 2 + sp) * 128:(g * 2 + sp + 1) * 128] = \
                    fsT_f8[:, x0:x0 + 128]
                hx[:, (g * 4 + st) * C:(g * 4 + st + 1) * C] = hot[x0:x0 + 128]
            for q in range(w):
                bs = BOFF[g] + q
                blk8 = fsT_f8[:, (c0 + q) * BLK:(c0 + q + 1) * BLK]
                ftrp[0:64, bs * BLK:(bs + 1) * BLK] = blk8
                ftrp[64:128, bs * BLK:(bs + 1) * BLK] = blk8
            if g in MASK_GROUPS:
                ms = MASK_GROUPS[g]
                # device stt window covers sim cols [lo, lo+MS_WIN); its
                # column j maps to sorted column (c0-block base + lo + j)
                lo = BLK if w == 3 else 0
                base = c0 * BLK + lo
                colab = labs_bf[base:base + MS_WIN[ms]].copy()
                for cb in range(c0, c0 + 3):
                    # zero out diag & non-near blocks inside the window
                    if not (cb > r and cb <= r + 2 and cb < c0 + w + (1 if w == 3 else 0)):
                        s = max(cb * BLK, base) - base
                        e = min((cb + 1) * BLK, base + MS_WIN[ms]) - base
                        if s < e:
                            colab[s:e] = -2.0
                labrep[:, MS_OFF[ms]:MS_OFF[ms] + MS_WIN[ms]] = colab[None, :]
                for st in range(4):
                    x0 = r * BLK + st * 128
                    labrow[:, ms * 4 + st] = labs_f[x0:x0 + 128]
        r0 = (k * I_PER_CORE) % N
        # rows r0..r0+1024 as 8 tiles of [128 rows, 64 D], tiles along cols
        a_pack = f1_bf[r0:r0 + I_PER_CORE].reshape(8, 128, 64) \
            .transpose(1, 0, 2).reshape(128, 8 * 64)
        b_pack = f2_bf[r0:r0 + I_PER_CORE].reshape(8, 128, 64) \
            .transpose(1, 0, 2).reshape(128, 8 * 64)
        in_maps.append({
            "ftl": ftl, "ftr": ftrp, "hx": hx,
            "labrep": labrep, "labrow": labrow,
            "a_i": np.ascontiguousarray(a_pack),
            "b_i": np.ascontiguousarray(b_pack),
        })

    res = run_bass_kernel_spmd(nc, in_maps, core_ids=list(range(NCORES)))
    _CACHE["last_res"] = res

    # ---- host epilogue ----
    acc = np.zeros((TWO_N, C), dtype=np.float64)   # per-class column sums
    tot_p = np.zeros(TWO_N, dtype=np.float64)      # row-total partials
    tot_diag = np.zeros(TWO_N, dtype=np.float64)   # diag double-count
    same_p = np.zeros(TWO_N, dtype=np.float64)
    dots = np.zeros(TWO_N, dtype=np.float64)
    for k in range(NCORES):
        r_ = res.results[k]
        segs = cores[k]
        acc_o = r_["acc_out"].astype(np.float64)
        tot_o = r_["tot_out"].astype(np.float64)
        same_o = r_["same_out"].astype(np.float64)
        isl = slice(k * I_PER_CORE, (k + 1) * I_PER_CORE)
        dots[isl] = r_["dots_out"].astype(np.float64).T.reshape(-1)
        for g, (r, c0, w) in enumerate(segs):
            for q in range(w):
                csl = slice((c0 + q) * BLK, (c0 + q + 1) * BLK)
                a_gq = acc_o[32 * q:32 * q + 16, g * BLK:(g + 1) * BLK]
                acc[csl] += a_gq.T
                if c0 + q == r:
                    # diag block: its columns are already in tot_p (ACT
                    # accum) AND in acc -- subtract one copy from tot
                    tot_diag[csl] += a_gq.sum(axis=0)
            for st in range(4):
                x0 = r * BLK + st * 128
                tot_p[x0:x0 + 128] += tot_o[:, g * 4 + st]
            if g in MASK_GROUPS:
                ms = MASK_GROUPS[g]
                for st in range(4):
                    x0 = r * BLK + st * 128
                    same_p[x0:x0 + 128] += same_o[:, ms * 4 + st]

    tot = acc.sum(axis=1) + tot_p - tot_diag
    same = acc[np.arange(TWO_N), labs] + same_p
    ng_sorted = tot - same
    ng = np.empty(TWO_N, dtype=np.float64)
    ng[perm] = ng_sorted
    dots[N:] = dots[:N]   # cores 4-7 recomputed the f1-half dots
    return _finish(ng, dots, label, lab2)


class _FallbackToV1(Exception):
    pass


def _finish(ng, dots, label, lab2):
    logpos = SCALE * dots
    pos = np.exp(logpos)
    term = np.log(ng + pos) - logpos
    counts = np.bincount(label, minlength=C)
    group_size = 2.0 * counts[lab2]
    loss = np.sum(term / group_size)
    return np.float32(loss)


# ---------------------------------------------------------------------------
# non-symmetric (v1) fallback: every core computes its 1024 columns against
# all 8192 rows; per-class sums via the H-matmul alone.
# ---------------------------------------------------------------------------

V1_NB_J = TWO_N // JSUB        # 64
V1_NB_I = I_PER_CORE // BLK    # 2
ACT_GROUP = 3


def _build_v1():
    nc = bacc.Bacc("TRN2", target_bir_lowering=False, debug=False,
                   num_devices=NCORES)
    ft_all = nc.declare_dram_parameter("ft_all", [D, TWO_N], BF16, isOutput=False)
    ft_i = nc.declare_dram_parameter("ft_i", [D, I_PER_CORE], BF16, isOutput=False)
    h_all = nc.declare_dram_parameter("h_all", [JSUB, V1_NB_J * C], BF16, isOutput=False)
    a_i = nc.declare_dram_parameter("a_i", [D, I_PER_CORE], F32, isOutput=False)
    b_i = nc.declare_dram_parameter("b_i", [D, I_PER_CORE], F32, isOutput=False)
    acc_out = nc.declare_dram_parameter("acc_out", [C, I_PER_CORE], F32, isOutput=True)
    dots_out = nc.declare_dram_parameter("dots_out", [1, I_PER_CORE], F32, isOutput=True)

    stream = [(ib, js) for ib in range(V1_NB_I) for js in range(V1_NB_J)]
    n_sub = len(stream)

    with tile.TileContext(nc) as tc:
        with (
            tc.tile_pool(name="consts", bufs=1) as consts,
            tc.tile_pool(name="simpool", bufs=4) as simpool,
            tc.tile_pool(name="small", bufs=2) as small,
            tc.tile_pool(name="plog", bufs=2, space="PSUM") as plog,
            tc.tile_pool(name="pacc", bufs=2, space="PSUM") as pacc,
        ):
            ft_all_sb = consts.tile([D, TWO_N], BF16)
            nc.sync.dma_start(out=ft_all_sb, in_=ft_all[:])
            ft_i_sb = consts.tile([D, I_PER_CORE], BF16)
            nc.sync.dma_start(out=ft_i_sb, in_=ft_i[:])
            h_sb = consts.tile([JSUB, V1_NB_J * C], BF16)
            nc.sync.dma_start(out=h_sb, in_=h_all[:])
            a_sb = consts.tile([D, I_PER_CORE], F32)
            nc.sync.dma_start(out=a_sb, in_=a_i[:])
            b_sb = consts.tile([D, I_PER_CORE], F32)
            nc.sync.dma_start(out=b_sb, in_=b_i[:])
            ones_sb = consts.tile([D, 1], F32)
            nc.vector.memset(ones_sb, 1.0)

            acc_sb = consts.tile([C, I_PER_CORE], F32)
            dots_sb = consts.tile([1, I_PER_CORE], F32)

            acc_ps = None
            g0 = 0
            while g0 < n_sub:
                gsz = min(ACT_GROUP, n_sub - g0)
                lg = plog.tile([JSUB, ACT_GROUP * BLK], F32, tag="lg")
                for u in range(gsz):
                    ib, js = stream[g0 + u]
                    nc.tensor.matmul(
                        lg[:, u * BLK:(u + 1) * BLK],
                        ft_all_sb[:, js * JSUB:(js + 1) * JSUB],
                        ft_i_sb[:, ib * BLK:(ib + 1) * BLK],
                        start=True, stop=True,
                    )
                sim = simpool.tile([JSUB, ACT_GROUP * BLK], BF16, tag="sim")
                nc.scalar.activation(
                    out=sim[:, :gsz * BLK],
                    in_=lg[:, :gsz * BLK],
                    func=mybir.ActivationFunctionType.Exp,
                    scale=SCALE,
                )
                for u in range(gsz):
                    ib, js = stream[g0 + u]
                    if js == 0:
                        acc_ps = pacc.tile([C, BLK], F32, tag="acc")
                    nc.tensor.matmul(
                        acc_ps,
                        h_sb[:, js * C:(js + 1) * C],
                        sim[:, u * BLK:(u + 1) * BLK],
                        start=(js == 0), stop=(js == V1_NB_J - 1),
                    )
                    if js == V1_NB_J - 1:
                        nc.vector.tensor_copy(
                            acc_sb[:, ib * BLK:(ib + 1) * BLK], acc_ps)
                g0 += gsz

            for ib in range(V1_NB_I):
                sl = slice(ib * BLK, (ib + 1) * BLK)
                prod = small.tile([D, BLK], F32, tag="prod")
                nc.vector.tensor_mul(prod, a_sb[:, sl], b_sb[:, sl])
                dps = pacc.tile([1, BLK], F32, tag="acc")
                nc.tensor.matmul(dps, ones_sb, prod, start=True, stop=True)
                nc.vector.tensor_copy(dots_sb[:, sl], dps)

            nc.sync.dma_start(out=acc_out[:], in_=acc_sb)
            nc.sync.dma_start(out=dots_out[:], in_=dots_sb)

    nc.compile()
    return nc


def _kernel_v1(f1, f2, label):
    if "nc1" not in _CACHE:
        _CACHE["nc1"] = _build_v1()
    nc = _CACHE["nc1"]

    feats = np.concatenate([f1, f2], axis=0)
    lab2 = np.concatenate([label, label], axis=0)
    ft_bf = np.ascontiguousarray(feats.T).astype(ml_dtypes.bfloat16)
    f1t = np.ascontiguousarray(f1.T)
    f2t = np.ascontiguousarray(f2.T)

    h_pack = np.zeros((JSUB, V1_NB_J * C), dtype=ml_dtypes.bfloat16)
    eye = np.eye(C, dtype=np.float32)
    for js in range(V1_NB_J):
        rows = lab2[js * JSUB:(js + 1) * JSUB]
        h_pack[:, js * C:(js + 1) * C] = eye[rows].astype(ml_dtypes.bfloat16)

    in_maps = []
    for k in range(NCORES):
        isl = slice(k * I_PER_CORE, (k + 1) * I_PER_CORE)
        r0 = (k * I_PER_CORE) % N
        in_maps.append({
            "ft_all": ft_bf,
            "ft_i": np.ascontiguousarray(ft_bf[:, isl]),
            "h_all": h_pack,
            "a_i": np.ascontiguousarray(f1t[:, r0:r0 + I_PER_CORE]),
            "b_i": np.ascontiguousarray(f2t[:, r0:r0 + I_PER_CORE]),
        })

    res = run_bass_kernel_spmd(nc, in_maps, core_ids=list(range(NCORES)))
    _CACHE["last_res"] = res

    acc = np.zeros((C, TWO_N), dtype=np.float64)
    dots = np.zeros(TWO_N, dtype=np.float64)
    for k in range(NCORES):
        isl = slice(k * I_PER_CORE, (k + 1) * I_PER_CORE)
        acc[:, isl] = res.results[k]["acc_out"].astype(np.float64)
        dots[isl] = res.results[k]["dots_out"][0].astype(np.float64)

    tot = acc.sum(axis=0)
    same = acc[lab2, np.arange(TWO_N)]
    ng = tot - same
    return _finish(ng, dots, label, lab2)


def kernel(f1, f2, label):
    f1 = np.asarray(f1, dtype=np.float32)
    f2 = np.asarray(f2, dtype=np.float32)
    label = np.asarray(label).astype(np.int64)
    try:
        return _kernel_v3(f1, f2, label)
    except _FallbackToV1:
        return _kernel_v1(f1, f2, label)

